# revision 8
# baseline (speedup 1.0000x reference)
"""ASTGCN head on 8 Trainium2 NeuronCores.

Key algebraic fact: with identity adjacency the Chebyshev stack is
[I, -I, I], so A[k,b] = +/- diag(diag_part(S[b])) and the graph conv
collapses to gcn[b,m,o,t] = relu(S[b,m,m] * (x @ (Th0-Th1+Th2))[b,m,o,t]).

Device (per core = (batch b, node-half j), two SPMD launches, one per block):
  - Spre = sVs^T-permuted @ sigmoid-matrix  (the B*N^3 dominant matmul)
  - unstable-softmax column sums (exp / column-sum via PE ones-matmul)
  - diagonal extraction via identity masks, S_diag = diag * 1/denom
  - gcn/res streams: per-t stationary x-slices against [Theta_eff | rcw]
Host: tiny temporal/spatial attention glue (T=12 matrices), 1x3 time conv,
LayerNorm, final FC, sharding/gather.  All fp32.
"""

import os
import sys
from contextlib import ExitStack

import numpy as np

if "/opt/trn_rl_repo" not in sys.path:
    sys.path.insert(0, "/opt/trn_rl_repo")

import concourse.bass as bass
import concourse.bacc as bacc
import concourse.tile as tile
from concourse import mybir
from concourse.bass_utils import run_bass_kernel_spmd

B, N, T, D, CC, KCH, PRED = 4, 1000, 12, 128, 64, 3, 12
F = 128          # padded feature dim (block1 uses 64, zero-padded)
NL = 512         # local nodes per core (j=1 has 488 real + 24 pad)
NCH = 8          # row chunks of the 1024-padded n dimension
MCH = 8          # contraction chunks: 1000 = 8 * 125
MC = 125
FP32 = mybir.dt.float32


def _build_nc():
    nc = bacc.Bacc("TRN2", target_bir_lowering=False, debug=False, num_devices=8)
    # all inputs packed by partition count -> one DMA (one sem) per tensor
    in125 = nc.dram_tensor("in125", [MC, 12288], FP32, kind="ExternalInput")
    in128 = nc.dram_tensor("in128", [128, 8321], FP32, kind="ExternalInput")
    out_gr = nc.dram_tensor("out_gr", [NL, T * 128], FP32, kind="ExternalOutput")

    with tile.TileContext(nc) as tc, ExitStack() as ctx:
        const_p = ctx.enter_context(tc.tile_pool(name="const", bufs=1))
        i125_p = ctx.enter_context(tc.tile_pool(name="i125", bufs=1))
        exp_p = ctx.enter_context(tc.tile_pool(name="expp", bufs=NCH))
        rec_p = ctx.enter_context(tc.tile_pool(name="recp", bufs=4))
        sd_p = ctx.enter_context(tc.tile_pool(name="sdp", bufs=4))
        scratch_p = ctx.enter_context(tc.tile_pool(name="scratch", bufs=2))
        out_p = ctx.enter_context(tc.tile_pool(name="outp", bufs=2))
        spre_ps = ctx.enter_context(tc.tile_pool(name="spre_ps", bufs=2, space="PSUM"))
        d_ps = ctx.enter_context(tc.tile_pool(name="d_ps", bufs=2, space="PSUM"))
        o_ps = ctx.enter_context(tc.tile_pool(name="o_ps", bufs=1, space="PSUM"))

        t125 = i125_p.tile([MC, 12288], FP32)
        nc.sync.dma_start(t125[:], in125[:])
        t128 = const_p.tile([128, 8321], FP32)
        nc.sync.dma_start(t128[:], in128[:])

        def sv(mc_, nch):
            o = mc_ * 1024 + nch * 128
            return t125[:, o : o + 128]

        def sg(mc_):
            return t125[:, 8192 + mc_ * NL : 8192 + (mc_ + 1) * NL]

        def xsl(t, mc_):
            o = t * NL + mc_ * 128
            return t128[:, o : o + 128]

        th_t = t128[:, 6144:6272]
        dm = lambda mc_: t128[:, 6272 + mc_ * NL : 6272 + (mc_ + 1) * NL]
        ones_t = t128[:, 8320:8321]

        def tail_mm(mc_):
            ops = o_ps.tile([128, T, 128], FP32, tag="ops")
            for t in range(T):
                nc.tensor.matmul(ops[:, t, :], xsl(t, mc_), th_t,
                                 start=True, stop=True)
            return ops

        def tail_post(mc_, ops, sd):
            ot = out_p.tile([128, T, 128], FP32, tag="ot")
            nc.scalar.activation(ot[:, :, 0:64], ops[:, :, 0:64],
                                 mybir.ActivationFunctionType.Relu, scale=sd[:])
            nc.scalar.copy(ot[:, :, 64:128], ops[:, :, 64:128])
            nc.sync.dma_start(out_gr[mc_ * 128 : (mc_ + 1) * 128, :],
                              ot.rearrange("p a b -> p (a b)"))

        # mc=0 tail matmuls first: PE's only wait is the in128 DMA sem
        ops0 = tail_mm(0)

        # ---- Spre = svst.T @ sig, per n-chunk; exp to SBUF ----
        exps = []
        for nch in range(NCH):
            ps = spre_ps.tile([128, NL], FP32, tag="ps")
            for mc_ in range(MCH):
                nc.tensor.matmul(ps[:], sv(mc_, nch), sg(mc_),
                                 start=(mc_ == 0), stop=(mc_ == MCH - 1))
            e = exp_p.tile([128, NL], FP32, tag="expS")
            nc.scalar.activation(e[:], ps[:], mybir.ActivationFunctionType.Exp)
            exps.append(e)

        # DVE prime: make DVE observe the in128 DMA with a lone 1-wait op
        prime1 = scratch_p.tile([128, 1], FP32, tag="prime1")
        nc.vector.tensor_copy(prime1[:], ones_t)

        # ---- column sums (transposed): denomT[ks] = sum_n exp(Spre[n,k]) ----
        recips = []
        for ks in range(4):
            dps = d_ps.tile([128, 1], FP32, tag="dps")
            for nch in range(NCH):
                nc.tensor.matmul(dps[:], exps[nch][:, ks * 128 : (ks + 1) * 128],
                                 ones_t, start=(nch == 0), stop=(nch == NCH - 1))
            dsb = scratch_p.tile([128, 1], FP32, tag="dsb")
            nc.vector.tensor_copy(dsb[:], dps[:])
            rec = rec_p.tile([128, 1], FP32, tag="rec")
            nc.vector.reciprocal(rec[:], dsb[:])
            recips.append(rec)

        # ---- diagonal of softmax for the 4 local chunks ----
        sdiags = []
        for mc_ in range(4):
            tmp = scratch_p.tile([128, NL], FP32, tag="dtmp")
            nc.vector.tensor_mul(tmp[:], exps[mc_][:], dm(mc_))
            dpart = scratch_p.tile([128, 1], FP32, tag="dpart")
            nc.vector.reduce_sum(dpart[:], tmp[:], axis=mybir.AxisListType.X)
            sd = sd_p.tile([128, 1], FP32, tag="sd")
            nc.vector.tensor_mul(sd[:], dpart[:], recips[mc_][:])
            sdiags.append(sd)

        # ACT prime: observe all DVE sdiag writes with a lone 1-wait op
        prime2 = scratch_p.tile([128, 1], FP32, tag="prime2")
        nc.scalar.copy(prime2[:], sdiags[3][:])

        tail_post(0, ops0, sdiags[0])
        for mc_ in range(1, 4):
            ops = tail_mm(mc_)
            tail_post(mc_, ops, sdiags[mc_])
    nc.compile()
    return nc


_NC_CACHE = None


def _get_nc():
    global _NC_CACHE
    if _NC_CACHE is None:
        _NC_CACHE = _build_nc()
    return _NC_CACHE


def _softmax0(a):
    a = a - a.max(axis=0, keepdims=True)
    e = np.exp(a)
    return e / e.sum(axis=0, keepdims=True)


def _sigmoid(a):
    return 1.0 / (1.0 + np.exp(-a))


def _host_glue(xf, tU1, tU2, tU3, tbe, tVe, sW1, sW2, sW3, sbs):
    """xf: (N,Fr,T) one batch. Returns sig (N,N) float32."""
    lhs = np.einsum("nft,n->tf", xf, tU1, optimize=True) @ tU2          # (T,N)
    rhs_t = np.einsum("f,nft->nt", tU3, xf, optimize=True)              # (N,T)
    inner = lhs @ rhs_t                                                  # (T,T)
    E = tVe @ _sigmoid(inner + tbe[0])                                   # (T,T)
    E = _softmax0(E)
    x_t = np.einsum("nft,ts->nfs", xf, E, optimize=True)                 # (N,Fr,T)
    slhs = np.einsum("nfs,s->nf", x_t, sW1, optimize=True) @ sW2         # (N,T)
    srhs = np.einsum("f,nfs->ns", sW3, x_t, optimize=True)               # (N,T)
    Pm = slhs @ srhs.T                                                   # (N,N)
    return _sigmoid(Pm + sbs[0]).astype(np.float32)


def _prep_core_inputs(sig_b, sVsT, xfn_b, thrc, j):
    n0 = j * NL
    nreal = NL if j == 0 else N - NL
    # svst columns: [local block | pad | remote block | pad], pad = -1 (exp->0)
    loc = sVsT[:, n0 : n0 + nreal]
    rem = np.concatenate([sVsT[:, :n0], sVsT[:, n0 + nreal :]], axis=1)
    padl = -np.ones((N, NL - nreal), np.float32)
    padr = -np.ones((N, 512 - rem.shape[1]), np.float32)
    sv = np.concatenate([loc, padl, rem, padr], axis=1)                  # (1000,1024)
    sv = sv.reshape(MCH, MC, 1024).transpose(1, 0, 2).reshape(MC, MCH * 1024)

    sg = np.zeros((N, NL), np.float32)
    sg[:, :nreal] = sig_b[:, n0 : n0 + nreal]
    sg = sg.reshape(MCH, MC, NL).transpose(1, 0, 2).reshape(MC, MCH * NL)

    xc = np.zeros((F, T, NL), np.float32)
    xc[: xfn_b.shape[0], :, :nreal] = xfn_b[:, :, n0 : n0 + nreal]

    dmk = np.zeros((128, 4 * NL), np.float32)
    idx = np.arange(512)
    valid = idx < nreal
    p = idx % 128
    mc_ = idx // 128
    dmk[p[valid], mc_[valid] * NL + idx[valid]] = 1.0

    i125 = np.concatenate([sv, sg], axis=1).astype(np.float32)
    i128 = np.concatenate(
        [xc.reshape(F, T * NL), thrc, dmk, np.ones((128, 1), np.float32)], axis=1
    ).astype(np.float32)
    return {"in125": np.ascontiguousarray(i125), "in128": np.ascontiguousarray(i128)}


def _run_block(xf_all, prm, results_to_h):
    """xf_all: (B,N,Fr,T); prm: dict of block params. Returns h (B,N,CC,T)."""
    nc = _get_nc()
    Fr = xf_all.shape[2]
    theta_eff = (prm["Theta"][0] - prm["Theta"][1] + prm["Theta"][2]).astype(np.float32)
    thrc = np.zeros((F, 128), np.float32)
    thrc[:Fr, :64] = theta_eff
    thrc[:Fr, 64:] = prm["rcw"][:, :, 0, 0].T
    sVsT = np.ascontiguousarray(prm["sVs"].T).astype(np.float32)

    in_maps = []
    for b in range(B):
        sig_b = _host_glue(
            xf_all[b], prm["tU1"], prm["tU2"], prm["tU3"], prm["tbe"], prm["tVe"],
            prm["sW1"], prm["sW2"], prm["sW3"], prm["sbs"],
        )
        xfn_b = np.ascontiguousarray(xf_all[b].transpose(1, 2, 0))  # (Fr,T,N)
        for j in range(2):
            in_maps.append(_prep_core_inputs(sig_b, sVsT, xfn_b, thrc, j))

    res = run_bass_kernel_spmd(nc, in_maps, list(range(8))).results

    h = np.zeros((B, N, CC, T), np.float32)
    tcw3 = prm["tcw"][:, :, 0, :]                                   # (O,C,3)
    for b in range(B):
        g0 = res[2 * b]["out_gr"].reshape(NL, T, 128)
        g1 = res[2 * b + 1]["out_gr"].reshape(NL, T, 128)
        gr = np.concatenate([g0, g1[: N - NL]], axis=0)             # (N,T,128)
        gcn, rsd = gr[:, :, :64], gr[:, :, 64:]
        gp = np.pad(gcn, ((0, 0), (1, 1), (0, 0)))
        W = np.stack([gp[:, 0:T], gp[:, 1 : T + 1], gp[:, 2 : T + 2]], axis=2)
        tc_ = np.einsum("ntdc,ocd->nto", W, tcw3, optimize=True)
        y = np.maximum(rsd + prm["rcb"] + tc_ + prm["tcb"], 0.0)
        mu = y.mean(-1, keepdims=True)
        var = y.var(-1, keepdims=True)
        y = (y - mu) / np.sqrt(var + 1e-5) * prm["lng"] + prm["lnb"]
        h[b] = y.transpose(0, 2, 1)                                  # (N,CC,T)
    return h


def _block_params(inputs, i):
    s = f"_{i}"
    keys = ["tU1", "tU2", "tU3", "tbe", "tVe", "sW1", "sW2", "sW3", "sbs",
            "sVs", "Theta", "tcw", "tcb", "rcw", "rcb", "lng", "lnb"]
    return {k: np.asarray(inputs[k + s], np.float32) for k in keys}


def kernel(**inputs):
    x = np.asarray(inputs["x"], np.float32)
    xf = np.ascontiguousarray(x.transpose(0, 1, 3, 2))               # (B,N,D,T)
    h = _run_block(xf, _block_params(inputs, 0), None)
    h2 = _run_block(h, _block_params(inputs, 1), None)
    fcw = np.asarray(inputs["fcw"], np.float32)[:, :, 0, :]          # (P,T,CC)
    fcb = np.asarray(inputs["fcb"], np.float32)
    out = np.einsum("bnft,ptf->bnp", h2, fcw, optimize=True) + fcb
    return out.astype(np.float32)
